# revision 1
# baseline (speedup 1.0000x reference)
"""LNN / echo-state step on 8 TRN2 NeuronCores — fp8 DoubleRow version.

Computes state = 0.7*prev_state + 0.3*tanh(inputs @ Wi^T + prev_state @ Wr^T)
for B=8192, IN=2048, R=4096 (fp32 in/out).

Strategy: data-parallel over batch (1024 rows/core, replicated weights), with
the two matmuls fused into one K=6144 contraction, computed in fp8 e4m3 with
perf_mode=DoubleRow (2x the fp32r/bf16 PE rate).

Quantization: operands are scaled by a power of two into e4m3's sweet spot
(x,h by 16; W by 64) on the host; the 1/1024 descale is folded into the tanh
activation's scale argument on ScalarE.  The 0.7*prev_state leak term uses a
bf16 copy of h pre-scaled by 0.7 on the host, so only the tanh argument sees
fp8 noise (total rel err ~1.2e-2 measured vs fp32 on CPU).  Output is written
bf16 and upcast on the host (adds ~0.2% rms, negligible vs the fp8 noise).

DMA queue split (per-core traffic 47MB @ ~360GB/s aggregate):
  - weights fp8 (25MB) stream on the SP HW-DGE queue,
  - activations fp8 (6MB, front-loaded) + out bf16 (8MB) on the Activation
    HW-DGE queue,
  - h07 bf16 (8MB) on the Pool SW-DGE queue,
so the weight stream — which feeds the PE — never waits behind epilogue
traffic on a single queue.

Per-core layout:
  - acts fp8 [24, 128, 2*1024]: pair-packed transposed activations
    (concat(x^T, h^T) quantized; pair j holds k-slabs 2j, 2j+1 side by side
    along the free dim so a [128, 2, n] moving AP is a strided slice).
  - wts fp8 [32, nchunk, 128, ktc*2*128]: per-output-m-tile weight chunks,
    pair-major so lhsT [128, 2, 128] slices are contiguous.
  - h07 bf16 [32, 128, 1024]: 0.7 * h^T, streamed per m-tile for the blend.
  - out bf16 [32, 128, 1024].
"""

import numpy as np
import ml_dtypes

import concourse.bass as bass
import concourse.mybir as mybir
from concourse import bacc
from concourse.tile import TileContext

P = 128
B_FULL, IN_DIM, R_DIM = 8192, 2048, 4096
N_CORES = 8
B_SHARD = B_FULL // N_CORES
LEAK = 0.3
SX = 16.0           # activation quantization scale (power of two)
SW = 64.0           # weight quantization scale (power of two)
KT2 = (IN_DIM + R_DIM) // (2 * P)   # 24 k-pairs (K=256 each)
MT = R_DIM // P                     # 32 output row tiles
N_TILE = 256                        # moving free per DoubleRow matmul
NT = B_SHARD // N_TILE              # 4

F8 = ml_dtypes.float8_e4m3
BF16 = ml_dtypes.bfloat16


def build_program(ktc=4, reps=1):
    """Emit the per-core Bass program. ktc = k-pairs per weight DMA chunk.

    reps > 1 wraps the whole body in a hardware For_i loop that re-runs the
    identical computation; used only for timing (one dispatch = reps kernel
    executions, amortizing the ~1.5-3ms axon dispatch overhead that would
    otherwise swamp the measurement)."""
    nchunk = KT2 // ktc
    assert KT2 % ktc == 0

    f8 = mybir.dt.float8e4
    f32 = mybir.dt.float32
    bf16 = mybir.dt.bfloat16
    Tanh = mybir.ActivationFunctionType.Tanh
    DR = mybir.MatmulPerfMode.DoubleRow

    nc = bacc.Bacc("TRN2", target_bir_lowering=False, debug=False)

    acts_d = nc.dram_tensor("acts", [KT2, P, 2 * B_SHARD], f8, kind="ExternalInput")
    wts_d = nc.dram_tensor("wts", [MT, nchunk, P, ktc * 2 * P], f8, kind="ExternalInput")
    h_d = nc.dram_tensor("h07", [MT, P, B_SHARD], bf16, kind="ExternalInput")
    out_d = nc.dram_tensor("out", [MT, P, B_SHARD], bf16, kind="ExternalOutput")

    with TileContext(nc) as tc:
        with (
            tc.tile_pool(name="act_pool", bufs=KT2) as apool,
            tc.tile_pool(name="w_pool", bufs=4) as wpool,
            tc.tile_pool(name="h_pool", bufs=2) as hpool,
            tc.tile_pool(name="t_pool", bufs=4) as tpool,
            tc.tile_pool(name="o_pool", bufs=4) as opool,
            tc.tile_pool(name="ps_pool", bufs=8, space="PSUM") as pspool,
        ):
            def body():
                act_tiles = []
                for j in range(KT2):
                    at = apool.tile([P, 2 * B_SHARD], f8, tag="act", name=f"act{j}")
                    nc.scalar.dma_start(at[:], acts_d[j])
                    act_tiles.append(at.rearrange("p (two b) -> p two b", two=2))

                for m in range(MT):
                    ht = hpool.tile([P, B_SHARD], bf16, tag="h")
                    nc.gpsimd.dma_start(ht[:], h_d[m])
                    psums = [pspool.tile([P, N_TILE], f32, tag="ps", name=f"ps{m}_{n}")
                             for n in range(NT)]
                    for ch in range(nchunk):
                        wc = wpool.tile([P, ktc * 2 * P], f8, tag="w")
                        nc.sync.dma_start(wc[:], wts_d[m, ch])
                        wcv = wc.rearrange("p (k two m) -> p k two m", k=ktc, two=2)
                        for jl in range(ktc):
                            j = ch * ktc + jl
                            lhsT = wcv[:, jl]
                            for n in range(NT):
                                rhs = act_tiles[j][:, :, n * N_TILE:(n + 1) * N_TILE]
                                nc.tensor.matmul(
                                    psums[n][:],
                                    lhsT,
                                    rhs,
                                    start=(j == 0),
                                    stop=(j == KT2 - 1),
                                    perf_mode=DR,
                                )
                    for n in range(NT):
                        t = tpool.tile([P, N_TILE], f32, tag="t")
                        nc.scalar.activation(t[:], psums[n][:], Tanh,
                                             scale=1.0 / (SX * SW))
                        o = opool.tile([P, N_TILE], bf16, tag="o")
                        nc.vector.scalar_tensor_tensor(
                            o[:], t[:], LEAK, ht[:, n * N_TILE:(n + 1) * N_TILE],
                            mybir.AluOpType.mult, mybir.AluOpType.add,
                        )
                        nc.scalar.dma_start(out_d[m, :, n * N_TILE:(n + 1) * N_TILE], o[:])

            if reps == 1:
                body()
            else:
                with tc.For_i(0, reps):
                    body()

    nc.compile()
    return nc


def pack_weights(input_weights, reservoir_weights, ktc=4):
    """[R, IN] + [R, R] fp32 -> [MT, nchunk, P, ktc*2*P] e4m3, pair-major."""
    w = np.concatenate(
        [np.ascontiguousarray(input_weights.T), np.ascontiguousarray(reservoir_weights.T)],
        axis=0,
    )  # [K, R] with K = IN + R
    wq = (w * SW).astype(F8)
    nchunk = KT2 // ktc
    # [m, ch, p, jl, i, mcol] = wq[((ch*ktc + jl)*2 + i)*P + p, m*P + mcol]
    wq = wq.reshape(nchunk, ktc, 2, P, MT, P).transpose(4, 0, 3, 1, 2, 5)
    return np.ascontiguousarray(wq.reshape(MT, nchunk, P, ktc * 2 * P))


def pack_acts(x_shard, h_shard):
    """[b, IN] + [b, R] fp32 -> [KT2, P, 2*b] e4m3, pair-packed."""
    a = np.concatenate([x_shard.T, h_shard.T], axis=0)  # [K, b]
    aq = (a * SX).astype(F8)
    b = aq.shape[1]
    # [j, p, i, n] = aq[(2j + i)*P + p, n]
    return np.ascontiguousarray(
        aq.reshape(KT2, 2, P, b).transpose(0, 2, 1, 3).reshape(KT2, P, 2 * b))


def make_in_maps(x, h, wi, wr):
    wts = pack_weights(wi, wr)
    in_maps = []
    for c in range(N_CORES):
        sl = slice(c * B_SHARD, (c + 1) * B_SHARD)
        h_sh = h[sl]
        h07 = np.ascontiguousarray(
            ((1.0 - LEAK) * h_sh.T.reshape(MT, P, B_SHARD)).astype(BF16))
        in_maps.append({
            "acts": pack_acts(x[sl], h_sh),
            "wts": wts,
            "h07": h07,
        })
    return in_maps


_CACHE = {}


def kernel(inputs, prev_state, input_weights, reservoir_weights):
    from concourse import bass_utils

    x = np.ascontiguousarray(np.asarray(inputs, dtype=np.float32))
    h = np.ascontiguousarray(np.asarray(prev_state, dtype=np.float32))
    wi = np.asarray(input_weights, dtype=np.float32)
    wr = np.asarray(reservoir_weights, dtype=np.float32)
    assert x.shape == (B_FULL, IN_DIM) and h.shape == (B_FULL, R_DIM)

    if "nc" not in _CACHE:
        _CACHE["nc"] = build_program()
    nc = _CACHE["nc"]

    in_maps = make_in_maps(x, h, wi, wr)
    res = bass_utils.run_bass_kernel_spmd(nc, in_maps, core_ids=list(range(N_CORES)))

    out = np.empty((B_FULL, R_DIM), dtype=np.float32)
    for c in range(N_CORES):
        o = res.results[c]["out"]  # [MT, P, B_SHARD] bf16
        out[c * B_SHARD:(c + 1) * B_SHARD] = \
            o.astype(np.float32).reshape(R_DIM, B_SHARD).T
    return out



# revision 2
# speedup vs baseline: 1.0546x; 1.0546x over previous
"""LNN / echo-state step on 8 TRN2 NeuronCores — fp8 DoubleRow version.

Computes state = 0.7*prev_state + 0.3*tanh(inputs @ Wi^T + prev_state @ Wr^T)
for B=8192, IN=2048, R=4096 (fp32 in/out).

Strategy: data-parallel over batch (1024 rows/core, replicated weights), with
the two matmuls fused into one K=6144 contraction, computed in fp8 e4m3 with
perf_mode=DoubleRow (2x the fp32r/bf16 PE rate).

Quantization: operands are scaled by a power of two into e4m3's sweet spot
(x,h by 16; W by 64) on the host; the 1/1024 descale is folded into the tanh
activation's scale argument on ScalarE.  The 0.7*prev_state leak term uses a
bf16 copy of h pre-scaled by 0.7 on the host, so only the tanh argument sees
fp8 noise (total rel err ~1.2e-2 measured vs fp32 on CPU).  Output is written
bf16 and upcast on the host (adds ~0.2% rms, negligible vs the fp8 noise).

DMA queue split (per-core traffic 47MB @ ~360GB/s aggregate):
  - weights fp8 (25MB) stream on the SP HW-DGE queue,
  - activations fp8 (6MB, front-loaded) + out bf16 (8MB) on the Activation
    HW-DGE queue,
  - h07 bf16 (8MB) on the Pool SW-DGE queue,
so the weight stream — which feeds the PE — never waits behind epilogue
traffic on a single queue.

Per-core layout:
  - acts fp8 [24, 128, 2*1024]: pair-packed transposed activations
    (concat(x^T, h^T) quantized; pair j holds k-slabs 2j, 2j+1 side by side
    along the free dim so a [128, 2, n] moving AP is a strided slice).
  - wts fp8 [32, nchunk, 128, ktc*2*128]: per-output-m-tile weight chunks,
    pair-major so lhsT [128, 2, 128] slices are contiguous.
  - h07 bf16 [32, 128, 1024]: 0.7 * h^T, streamed per m-tile for the blend.
  - out bf16 [32, 128, 1024].

Optimization status (second session): this shape is at the sustained
DoubleRow PE ceiling (~140 ns per [K=256 x N=256] DR matmul ~= 1.3
cyc/outcol, consistent with the documented ~1.5x-over-bf16 DR rate under
the P0 sustained-load clock).  Variants measured on HW and all within
noise or worse: DoubleRowSwInterleave weights (430us, bit-identical
numerics), N_TILE=512 (~435us), hybrid 4-batch x 2-R-half sharding with
2048 moving cols per LDWEIGHTS (470us — act-stream pacing stalls the
first m-tile and the For_i rep boundary), double-buffered act pool with
outs moved to the SP ring (434us).  LDWEIGHTS exposure, PSUM tiling, and
DMA-queue layout are all fully hidden behind the matmul stream; beating
~430us would need a faster compute mode than fp8 DoubleRow, which TRN2
does not offer.
"""

import numpy as np
import ml_dtypes

import concourse.bass as bass
import concourse.mybir as mybir
from concourse import bacc
from concourse.tile import TileContext

P = 128
B_FULL, IN_DIM, R_DIM = 8192, 2048, 4096
N_CORES = 8
B_SHARD = B_FULL // N_CORES
LEAK = 0.3
SX = 16.0           # activation quantization scale (power of two)
SW = 64.0           # weight quantization scale (power of two)
KT2 = (IN_DIM + R_DIM) // (2 * P)   # 24 k-pairs (K=256 each)
MT = R_DIM // P                     # 32 output row tiles
N_TILE = 256                        # moving free per DoubleRow matmul
NT = B_SHARD // N_TILE              # 4

F8 = ml_dtypes.float8_e4m3
BF16 = ml_dtypes.bfloat16


def build_program(ktc=4, reps=1):
    """Emit the per-core Bass program. ktc = k-pairs per weight DMA chunk.

    reps > 1 wraps the whole body in a hardware For_i loop that re-runs the
    identical computation; used only for timing (one dispatch = reps kernel
    executions, amortizing the ~1.5-3ms axon dispatch overhead that would
    otherwise swamp the measurement)."""
    nchunk = KT2 // ktc
    assert KT2 % ktc == 0

    f8 = mybir.dt.float8e4
    f32 = mybir.dt.float32
    bf16 = mybir.dt.bfloat16
    Tanh = mybir.ActivationFunctionType.Tanh
    DR = mybir.MatmulPerfMode.DoubleRow

    nc = bacc.Bacc("TRN2", target_bir_lowering=False, debug=False)

    acts_d = nc.dram_tensor("acts", [KT2, P, 2 * B_SHARD], f8, kind="ExternalInput")
    wts_d = nc.dram_tensor("wts", [MT, nchunk, P, ktc * 2 * P], f8, kind="ExternalInput")
    h_d = nc.dram_tensor("h07", [MT, P, B_SHARD], bf16, kind="ExternalInput")
    out_d = nc.dram_tensor("out", [MT, P, B_SHARD], bf16, kind="ExternalOutput")

    with TileContext(nc) as tc:
        with (
            tc.tile_pool(name="act_pool", bufs=KT2) as apool,
            tc.tile_pool(name="w_pool", bufs=4) as wpool,
            tc.tile_pool(name="h_pool", bufs=2) as hpool,
            tc.tile_pool(name="t_pool", bufs=4) as tpool,
            tc.tile_pool(name="o_pool", bufs=4) as opool,
            tc.tile_pool(name="ps_pool", bufs=8, space="PSUM") as pspool,
        ):
            def body():
                act_tiles = []
                for j in range(KT2):
                    at = apool.tile([P, 2 * B_SHARD], f8, tag="act", name=f"act{j}")
                    nc.scalar.dma_start(at[:], acts_d[j])
                    act_tiles.append(at.rearrange("p (two b) -> p two b", two=2))

                for m in range(MT):
                    ht = hpool.tile([P, B_SHARD], bf16, tag="h")
                    nc.gpsimd.dma_start(ht[:], h_d[m])
                    psums = [pspool.tile([P, N_TILE], f32, tag="ps", name=f"ps{m}_{n}")
                             for n in range(NT)]
                    for ch in range(nchunk):
                        wc = wpool.tile([P, ktc * 2 * P], f8, tag="w")
                        nc.sync.dma_start(wc[:], wts_d[m, ch])
                        wcv = wc.rearrange("p (k two m) -> p k two m", k=ktc, two=2)
                        for jl in range(ktc):
                            j = ch * ktc + jl
                            lhsT = wcv[:, jl]
                            for n in range(NT):
                                rhs = act_tiles[j][:, :, n * N_TILE:(n + 1) * N_TILE]
                                nc.tensor.matmul(
                                    psums[n][:],
                                    lhsT,
                                    rhs,
                                    start=(j == 0),
                                    stop=(j == KT2 - 1),
                                    perf_mode=DR,
                                )
                    for n in range(NT):
                        t = tpool.tile([P, N_TILE], f32, tag="t")
                        nc.scalar.activation(t[:], psums[n][:], Tanh,
                                             scale=1.0 / (SX * SW))
                        o = opool.tile([P, N_TILE], bf16, tag="o")
                        nc.vector.scalar_tensor_tensor(
                            o[:], t[:], LEAK, ht[:, n * N_TILE:(n + 1) * N_TILE],
                            mybir.AluOpType.mult, mybir.AluOpType.add,
                        )
                        nc.scalar.dma_start(out_d[m, :, n * N_TILE:(n + 1) * N_TILE], o[:])

            if reps == 1:
                body()
            else:
                with tc.For_i(0, reps):
                    body()

    nc.compile()
    return nc


def pack_weights(input_weights, reservoir_weights, ktc=4):
    """[R, IN] + [R, R] fp32 -> [MT, nchunk, P, ktc*2*P] e4m3, pair-major."""
    w = np.concatenate(
        [np.ascontiguousarray(input_weights.T), np.ascontiguousarray(reservoir_weights.T)],
        axis=0,
    )  # [K, R] with K = IN + R
    wq = (w * SW).astype(F8)
    nchunk = KT2 // ktc
    # [m, ch, p, jl, i, mcol] = wq[((ch*ktc + jl)*2 + i)*P + p, m*P + mcol]
    wq = wq.reshape(nchunk, ktc, 2, P, MT, P).transpose(4, 0, 3, 1, 2, 5)
    return np.ascontiguousarray(wq.reshape(MT, nchunk, P, ktc * 2 * P))


def pack_acts(x_shard, h_shard):
    """[b, IN] + [b, R] fp32 -> [KT2, P, 2*b] e4m3, pair-packed."""
    a = np.concatenate([x_shard.T, h_shard.T], axis=0)  # [K, b]
    aq = (a * SX).astype(F8)
    b = aq.shape[1]
    # [j, p, i, n] = aq[(2j + i)*P + p, n]
    return np.ascontiguousarray(
        aq.reshape(KT2, 2, P, b).transpose(0, 2, 1, 3).reshape(KT2, P, 2 * b))


def make_in_maps(x, h, wi, wr):
    wts = pack_weights(wi, wr)
    in_maps = []
    for c in range(N_CORES):
        sl = slice(c * B_SHARD, (c + 1) * B_SHARD)
        h_sh = h[sl]
        h07 = np.ascontiguousarray(
            ((1.0 - LEAK) * h_sh.T.reshape(MT, P, B_SHARD)).astype(BF16))
        in_maps.append({
            "acts": pack_acts(x[sl], h_sh),
            "wts": wts,
            "h07": h07,
        })
    return in_maps


_CACHE = {}


def kernel(inputs, prev_state, input_weights, reservoir_weights):
    from concourse import bass_utils

    x = np.ascontiguousarray(np.asarray(inputs, dtype=np.float32))
    h = np.ascontiguousarray(np.asarray(prev_state, dtype=np.float32))
    wi = np.asarray(input_weights, dtype=np.float32)
    wr = np.asarray(reservoir_weights, dtype=np.float32)
    assert x.shape == (B_FULL, IN_DIM) and h.shape == (B_FULL, R_DIM)

    if "nc" not in _CACHE:
        _CACHE["nc"] = build_program()
    nc = _CACHE["nc"]

    in_maps = make_in_maps(x, h, wi, wr)
    res = bass_utils.run_bass_kernel_spmd(nc, in_maps, core_ids=list(range(N_CORES)))

    out = np.empty((B_FULL, R_DIM), dtype=np.float32)
    for c in range(N_CORES):
        o = res.results[c]["out"]  # [MT, P, B_SHARD] bf16
        out[c * B_SHARD:(c + 1) * B_SHARD] = \
            o.astype(np.float32).reshape(R_DIM, B_SHARD).T
    return out

